# revision 75
# baseline (speedup 1.0000x reference)
"""Trainium2 Bass kernel for ModalityAttention (B=4, S=1024, D=2048, H=16, HD=128, RD=64).

Sharding: 8 cores = 4 batches x 2 head-groups (8 heads each).
Each core computes, for its (batch b, head-group g):
  layernorm(x[b]) -> modulation (scale/bias precomputed on host from mod@mod_w)
  -> qkv projection for its 8 heads -> rmsnorm + rope -> attention
  -> partial out-projection (transposed layout) with gate folded in.
Host gathers: out[b] = (partial_g0 + partial_g1).T + x[b]
(residual added on host; vb = out_b*gate folded into the g0 partial on device).

All matmul operands are bf16 (fp32 PSUM accumulation); LN/rms/rope/softmax
statistics stay fp32 where it matters.  q/k columns are host-permuted per
head so the rope pair halves are contiguous (inner products are invariant
to a shared permutation of q and k columns).  Emission interleaves phases
(A with B-q, C-q before B-k, C-k before B-v) to keep the in-order PE
stream dense.
"""
import os, sys

for _p in ("/opt/trn_rl_repo", "/root/.axon_site/_ro/trn_rl_repo", "/root/.axon_site"):
    if os.path.isdir(_p) and _p not in sys.path:
        sys.path.insert(0, _p)

import numpy as np
import ml_dtypes
import concourse.bass as bass
import concourse.bacc as bacc
import concourse.mybir as mybir
import concourse.tile as tile
from concourse import bass_isa
from concourse.masks import make_identity
from concourse.bass_utils import run_bass_kernel_spmd

F32 = mybir.dt.float32
BF16 = mybir.dt.bfloat16
AF = mybir.ActivationFunctionType
ALU = mybir.AluOpType
S, D, HG, HD, RD = 1024, 2048, 8, 128, 64
NT = S // 128        # 8 s-tiles
KT = D // 128        # 16 d-tiles
GCOLS = HG * HD      # 1024 columns per group per projection
EPS = 1e-6
N_CORES = 8


def _bcast_from_dram(ap, parts, reps=None):
    """DRAM AP -> partition-broadcast (and optional middle-dim repeat) source AP."""
    newap = [[0, parts]]
    if reps is not None:
        newap.append([0, reps])
    newap += list(ap.ap)
    return bass.AP(tensor=ap.tensor, offset=ap.offset, ap=newap)


def build_nc(has_qkv_bias: bool, has_norm_w: bool):
    nc = bacc.Bacc("TRN2", target_bir_lowering=False, debug=False,
                   enable_asserts=True, num_devices=N_CORES)

    x = nc.dram_tensor("x", [S, D], BF16, kind="ExternalInput").ap()
    cos = nc.dram_tensor("cos", [S, RD // 2], BF16, kind="ExternalInput").ap()
    sin = nc.dram_tensor("sin", [S, RD // 2], BF16, kind="ExternalInput").ap()
    wq = nc.dram_tensor("wq", [D, GCOLS], BF16, kind="ExternalInput").ap()
    wk = nc.dram_tensor("wk", [D, GCOLS], BF16, kind="ExternalInput").ap()
    wv = nc.dram_tensor("wv", [D, GCOLS], BF16, kind="ExternalInput").ap()
    # wo pre-laid-out on host: wo_l[p, m, kb, c] = wo[kb*128+p, m*128+c]
    wo = nc.dram_tensor("wo", [128, KT, HG, 128], BF16, kind="ExternalInput").ap()
    # modulation vectors, pre-reshaped on host to [128, KT] (column k = d-tile k)
    scale1p = nc.dram_tensor("scale1p", [128, KT], F32, kind="ExternalInput").ap()
    biasm = nc.dram_tensor("biasm", [128, KT], F32, kind="ExternalInput").ap()
    gate = nc.dram_tensor("gate", [128, KT], F32, kind="ExternalInput").ap()
    vb = nc.dram_tensor("vb", [128, KT], F32, kind="ExternalInput").ap()
    if has_qkv_bias:
        bq = nc.dram_tensor("bq", [GCOLS], F32, kind="ExternalInput").ap()
        bk = nc.dram_tensor("bk", [GCOLS], F32, kind="ExternalInput").ap()
        bv = nc.dram_tensor("bv", [GCOLS], F32, kind="ExternalInput").ap()
    if has_norm_w:
        wqn = nc.dram_tensor("wqn", [HD], BF16, kind="ExternalInput").ap()
        wkn = nc.dram_tensor("wkn", [HD], BF16, kind="ExternalInput").ap()
    out_t = nc.dram_tensor("out_t", [D, S], F32, kind="ExternalOutput").ap()

    with tile.TileContext(nc) as tc:
        # ======== LEFT stack bottom: small persistent constants ====================
        misc_cm = tc.tile_pool(name="misc", bufs=1, side="left")
        misc = misc_cm.__enter__()
        ident_bf = misc.tile([128, 128], BF16)
        make_identity(nc, ident_bf)
        eps_t = misc.tile([128, 1], F32)
        nc.vector.memset(eps_t, EPS)
        eps128_t = misc.tile([128, 1], F32)
        nc.vector.memset(eps128_t, HD * EPS)
        gate_sb = misc.tile([128, KT], F32)
        vb_sb = misc.tile([128, KT], F32)
        rrk_all = misc.tile([128, NT, HG], F32)   # scaled k-rms reciprocals
        if has_norm_w:
            wqn_b = misc.tile([128, HG, HD], BF16)
            wkn_b = misc.tile([128, HG, HD], BF16)
        cos_all = misc.tile([128, NT, RD // 2], BF16)
        sin_all = misc.tile([128, NT, RD // 2], BF16)
        cs_tiles = [(cos_all[:, m, :], sin_all[:, m, :]) for m in range(NT)]

        # ======== RIGHT stack: natural-layout q/k/v (bf16) =========================
        v_cm = tc.tile_pool(name="vpool", bufs=1, side="right")
        v_p = v_cm.__enter__()
        vnat = v_p.tile([128, NT, GCOLS], BF16)
        natqk_cm = tc.tile_pool(name="natqk", bufs=1, side="right")
        natqk = natqk_cm.__enter__()
        qnat = natqk.tile([128, NT, GCOLS], BF16)
        knat = natqk.tile([128, NT, GCOLS], BF16)

        # resident qkv weights, double-buffered across projections
        wres_cm = tc.tile_pool(name="wres", bufs=2, side="right")
        wres_p = wres_cm.__enter__()
        wres_t = [wres_p.tile([128, KT, GCOLS], BF16, tag="wres", name=f"wres{pi}")
                  for pi in range(3)]
        wq_r = wq.rearrange("(k p) c -> p k c", p=128)
        wk_r = wk.rearrange("(k p) c -> p k c", p=128)
        wv_r = wv.rearrange("(k p) c -> p k c", p=128)
        w_drams = (wq_r, wk_r, wv_r)

        def load_wres(pi, chunks=range(4)):
            for c4 in chunks:
                nc.sync.dma_start(
                    out=wres_t[pi][:, c4 * 4:(c4 + 1) * 4, :],
                    in_=w_drams[pi][:, c4 * 4:(c4 + 1) * 4, :])

        # ======== phase A pools ====================================================
        xnT_cm = tc.tile_pool(name="xnT", bufs=1, side="left")
        xnT_p = xnT_cm.__enter__()
        xnT = xnT_p.tile([128, KT, S], BF16)  # [d_in_tile, d_tile, s]

        avec_cm = tc.tile_pool(name="phA_vec", bufs=1, side="left")
        avec = avec_cm.__enter__()
        s1pc = avec.tile([128, KT], F32)
        bmc = avec.tile([128, KT], F32)
        if has_qkv_bias:
            bq_b = avec.tile([128, GCOLS], F32)
            nc.sync.dma_start(out=bq_b, in_=_bcast_from_dram(bq, 128))
            bk_b = avec.tile([128, GCOLS], F32)
            nc.sync.dma_start(out=bk_b, in_=_bcast_from_dram(bk, 128))
            bv_b = avec.tile([128, GCOLS], F32)
            nc.sync.dma_start(out=bv_b, in_=_bcast_from_dram(bv, 128))

        a_cm = tc.tile_pool(name="phA", bufs=3, side="left")
        a_p = a_cm.__enter__()
        a_small_cm = tc.tile_pool(name="phA_small", bufs=6, side="left")
        a_small = a_small_cm.__enter__()
        pst_cm = tc.tile_pool(name="ps_tr", bufs=5, space="PSUM")
        pst = pst_cm.__enter__()

        # phase C pools opened early so later emission can overlap B on DVE
        c_cm = tc.tile_pool(name="phC", bufs=2, side="left")
        c_p = c_cm.__enter__()
        c_small_cm = tc.tile_pool(name="phC_small", bufs=2, side="left")
        c_small = c_small_cm.__enter__()

        psb_cm = tc.tile_pool(name="ps_qkv", bufs=3, space="PSUM")
        psb = psb_cm.__enter__()

        # ---- emit helpers ----
        def emit_A_stats(i):
            """x load + LN statistics; emitted 2 s-tiles ahead of the body so
            the serial stats chain never gates the PE transposes."""
            xt = a_p.tile([128, D], BF16, tag="xt", name=f"xt{i}")
            nc.sync.dma_start(out=xt[:, 0:D // 2],
                              in_=x[i * 128:(i + 1) * 128, 0:D // 2])
            nc.sync.dma_start(out=xt[:, D // 2:D],
                              in_=x[i * 128:(i + 1) * 128, D // 2:D])
            if i == 0:
                nc.sync.dma_start(out=s1pc, in_=scale1p)
                nc.sync.dma_start(out=bmc, in_=biasm)
            stats = a_small.tile([128, 4, 6], F32, tag="stats",
                                 name=f"stats{i}")
            xv = xt.rearrange("p (c f) -> p c f", c=4)
            for c in range(4):
                nc.vector.bn_stats(out=stats[:, c, :], in_=xv[:, c, :])
            mv = a_small.tile([128, 2], F32, tag="mv", name=f"mv{i}")
            nc.vector.bn_aggr(out=mv, in_=stats)
            rstd = a_small.tile([128, 1], F32, tag="rstd", name=f"rstd{i}")
            nc.scalar.activation(out=rstd, in_=mv[:, 1:2], func=AF.Sqrt,
                                 bias=eps_t, scale=1.0)
            nc.vector.reciprocal(out=rstd, in_=rstd)
            nmr = a_small.tile([128, 1], F32, tag="nmr", name=f"nmr{i}")
            nc.vector.tensor_scalar(out=nmr, in0=mv[:, 0:1], scalar1=rstd,
                                    scalar2=-1.0, op0=ALU.mult, op1=ALU.mult)
            return xt, rstd, nmr

        def emit_A_body(i, xt, rstd, nmr):
            xnb = a_p.tile([128, D], BF16, tag="xnb")
            nc.scalar.activation(out=xnb[:, 0:D // 2], in_=xt[:, 0:D // 2],
                                 func=AF.Identity, bias=nmr, scale=rstd)
            nc.vector.tensor_scalar(out=xnb[:, D // 2:D], in0=xt[:, D // 2:D],
                                    scalar1=rstd, scalar2=nmr,
                                    op0=ALU.mult, op1=ALU.add)
            for k in range(KT):
                pt = pst.tile([128, 128], BF16, tag="pt")
                nc.tensor.transpose(pt, xnb[:, k * 128:(k + 1) * 128], ident_bf)
                # modulation fused into the evac: xnT = pt*(1+scale[d]) + bias[d]
                dst = xnT[:, k, i * 128:(i + 1) * 128]
                if k % 2 == 0:
                    nc.scalar.activation(out=dst, in_=pt, func=AF.Identity,
                                         bias=bmc[:, k:k + 1],
                                         scale=s1pc[:, k:k + 1])
                else:
                    nc.vector.tensor_scalar(out=dst, in0=pt,
                                            scalar1=s1pc[:, k:k + 1],
                                            scalar2=bmc[:, k:k + 1],
                                            op0=ALU.mult, op1=ALU.add)

        def emit_Bm(pi, nat, m, evac_engines):
            wt = wres_t[pi]
            for n in range(2):
                ps = psb.tile([128, 512], F32, tag="ps")
                for k in range(KT):
                    nc.tensor.matmul(ps, xnT[:, k, m * 128:(m + 1) * 128],
                                     wt[:, k, n * 512:(n + 1) * 512],
                                     start=(k == 0), stop=(k == KT - 1))
                dst = nat[:, m, n * 512:(n + 1) * 512]
                if evac_engines[n] == "act":
                    nc.scalar.copy(out=dst, in_=ps)
                else:
                    nc.vector.tensor_copy(out=dst, in_=ps)

        def emit_Cq(m):
            qm = qnat[:, m, :]
            qmh = qm.rearrange("p (h c) -> p h c", h=HG)
            (ct, st) = cs_tiles[m]
            cb = ct.unsqueeze(1).broadcast_to([128, HG, RD // 2])
            sb_ = st.unsqueeze(1).broadcast_to([128, HG, RD // 2])
            sq = c_p.tile([128, GCOLS], BF16, tag="sqk")
            nc.vector.tensor_mul(out=sq, in0=qm, in1=qm)
            ssq = c_small.tile([128, HG], F32, tag="ssq")
            nc.vector.reduce_sum(out=ssq,
                                 in_=sq.rearrange("p (h d) -> p h d", h=HG),
                                 axis=mybir.AxisListType.X)
            rrq = c_small.tile([128, HG], BF16, tag="rrq")
            nc.scalar.activation(out=rrq, in_=ssq, func=AF.Sqrt,
                                 bias=eps_t, scale=1.0 / HD)
            with nc.allow_low_precision(reason="bf16 1/rms scale; 0.4% rel"):
                nc.vector.reciprocal(out=rrq, in_=rrq)
            if has_norm_w:
                nc.vector.tensor_mul(out=qmh, in0=qmh, in1=wqn_b)
            _emit_rope(qmh, cb, sb_)
            rrq_b = rrq.unsqueeze(2).broadcast_to([128, HG, HD])
            nc.vector.tensor_mul(out=qmh, in0=qmh, in1=rrq_b)

        def emit_Ck(m):
            km = knat[:, m, :]
            kmh = km.rearrange("p (h c) -> p h c", h=HG)
            (ct, st) = cs_tiles[m]
            cb = ct.unsqueeze(1).broadcast_to([128, HG, RD // 2])
            sb_ = st.unsqueeze(1).broadcast_to([128, HG, RD // 2])
            sk_ = c_p.tile([128, GCOLS], BF16, tag="sqk")
            nc.vector.tensor_mul(out=sk_, in0=km, in1=km)
            ssk = c_small.tile([128, HG], F32, tag="ssk")
            nc.vector.reduce_sum(out=ssk,
                                 in_=sk_.rearrange("p (h d) -> p h d", h=HG),
                                 axis=mybir.AxisListType.X)
            nc.scalar.activation(out=rrk_all[:, m, :], in_=ssk, func=AF.Sqrt,
                                 bias=eps128_t, scale=1.0)
            nc.vector.reciprocal(out=rrk_all[:, m, :], in_=rrk_all[:, m, :])
            if has_norm_w:
                nc.vector.tensor_mul(out=kmh, in0=kmh, in1=wkn_b)
            _emit_rope(kmh, cb, sb_)

        def _emit_rope(mh, cb, sb_):
            # host-de-interleaved: cols [0:32]=x0, [32:64]=x1 per head
            x0 = mh[:, :, 0:RD // 2]
            x1 = mh[:, :, RD // 2:RD]
            t0 = c_small.tile([128, HG, RD // 2], BF16, tag="t0")
            t1 = c_small.tile([128, HG, RD // 2], BF16, tag="t1")
            t2 = c_small.tile([128, HG, RD // 2], BF16, tag="t2")
            t3 = c_small.tile([128, HG, RD // 2], BF16, tag="t3")
            nc.vector.tensor_mul(out=t0, in0=x0, in1=cb)
            nc.vector.tensor_mul(out=t1, in0=x1, in1=sb_)
            nc.vector.tensor_mul(out=t2, in0=x0, in1=sb_)
            nc.vector.tensor_mul(out=t3, in0=x1, in1=cb)
            nc.vector.tensor_sub(out=x0, in0=t0, in1=t1)
            nc.vector.tensor_add(out=x1, in0=t2, in1=t3)

        # ---- fused A + B-q emission ----
        pend = {0: emit_A_stats(0), 1: emit_A_stats(1)}
        for i in range(NT):
            if i + 2 < NT:
                pend[i + 2] = emit_A_stats(i + 2)
            emit_A_body(i, *pend.pop(i))
            # all wq chunks MUST be emitted before the first emit_Bm below
            # (deps only order reads after already-emitted writes)
            if i == 0:
                load_wres(0, range(0, 2))
            elif i == 1:
                load_wres(0, range(2, 4))
            elif i == 3:
                load_wres(1)          # wk, consumed after A+B-q
            if i >= 1:
                emit_Bm(0, qnat, i - 1, ("act", "dve"))
        emit_Bm(0, qnat, NT - 1, ("act", "dve"))

        # deferred misc loads (consumed in phases C/E/F) — single batched DMAs
        # so their dispatch overhead doesn't contend with the x/w streams
        nc.sync.dma_start(out=gate_sb, in_=gate)
        nc.sync.dma_start(out=vb_sb, in_=vb)
        if has_norm_w:
            nc.sync.dma_start(out=wqn_b, in_=_bcast_from_dram(wqn, 128, reps=HG))
            nc.sync.dma_start(out=wkn_b, in_=_bcast_from_dram(wkn, 128, reps=HG))
        nc.sync.dma_start(out=cos_all, in_=cos.rearrange("(m p) f -> p m f", p=128))
        nc.sync.dma_start(out=sin_all, in_=sin.rearrange("(m p) f -> p m f", p=128))

        if has_qkv_bias:
            for m in range(NT):
                nc.gpsimd.tensor_add(out=qnat[:, m, :], in0=qnat[:, m, :], in1=bq_b)

        # ---- C-q (DVE) overlapping B-k (PE) ----
        for m in range(NT):
            emit_Cq(m)
        for m in range(NT):
            emit_Bm(1, knat, m, ("act", "act"))
        load_wres(2)                  # wv
        if has_qkv_bias:
            for m in range(NT):
                nc.gpsimd.tensor_add(out=knat[:, m, :], in0=knat[:, m, :], in1=bk_b)
        for m in range(NT):
            emit_Ck(m)
        for m in range(NT):
            emit_Bm(2, vnat, m, ("act", "act"))
        if has_qkv_bias:
            for m in range(NT):
                nc.gpsimd.tensor_add(out=vnat[:, m, :], in0=vnat[:, m, :], in1=bv_b)

        psb_cm.__exit__(None, None, None)
        pst_cm.__exit__(None, None, None)
        c_small_cm.__exit__(None, None, None)
        c_cm.__exit__(None, None, None)
        a_small_cm.__exit__(None, None, None)
        a_cm.__exit__(None, None, None)
        avec_cm.__exit__(None, None, None)
        xnT_cm.__exit__(None, None, None)
        wres_cm.__exit__(None, None, None)

        # ======== phases D/E/F share the left stack ================================
        oT_cm = tc.tile_pool(name="oT", bufs=1, side="left")
        oT_p = oT_cm.__enter__()
        oT = oT_p.tile([128, HG, S], BF16)

        # wo prefetch (resident; consumed in phase F)
        wo_cm = tc.tile_pool(name="wo_res", bufs=1, side="left")
        wo_p = wo_cm.__enter__()
        wo_sb = wo_p.tile([128, KT, HG, 128], BF16)

        # ---- phase D: transpose q, k -> [hd, s] per head (batched evacs)
        qkT_cm = tc.tile_pool(name="qkT", bufs=1, side="left")
        qkT_p = qkT_cm.__enter__()
        qT = qkT_p.tile([128, HG, S], BF16)
        kT = qkT_p.tile([128, HG, S], BF16)
        pst2_cm = tc.tile_pool(name="ps_tr2", bufs=4, space="PSUM")
        pst2 = pst2_cm.__enter__()
        for (nat, dstT) in ((qnat, qT), (knat, kT)):
            for h in range(HG):
                for g in range(2):
                    pt4 = pst2.tile([128, 512], BF16, tag="pt4")
                    for j in range(4):
                        m = g * 4 + j
                        nc.tensor.transpose(pt4[:, j * 128:(j + 1) * 128],
                                            nat[:, m, h * 128:(h + 1) * 128],
                                            ident_bf)
                    nc.vector.tensor_copy(out=dstT[:, h, g * 512:(g + 1) * 512],
                                          in_=pt4)
        pst2_cm.__exit__(None, None, None)
        # (natqk stays open through E/F: closing it between D and E would
        # add a pool barrier; SBUF has room)

        # wo loads (consumed in F; DMA-engine time hidden under phase E)
        for m in range(KT):
            nc.sync.dma_start(out=wo_sb[:, m, :, :], in_=wo[:, m, :, :])

        # ---- phase E: attention per head
        at_cm = tc.tile_pool(name="attn", bufs=6, side="left")
        at_p = at_cm.__enter__()
        rs_cm = tc.tile_pool(name="rsb", bufs=4, side="left")
        rs_p = rs_cm.__enter__()
        pssc_cm = tc.tile_pool(name="ps_sc", bufs=2, space="PSUM")
        pssc = pssc_cm.__enter__()
        pso_cm = tc.tile_pool(name="ps_o", bufs=2, space="PSUM")
        pso = pso_cm.__enter__()

        def emit_scores(h, m):
            sc = pssc.tile([128, S], F32, tag="sc")
            lhs_k = kT[:, h, m * 128:(m + 1) * 128]
            nc.tensor.matmul(sc[:, 0:512], lhs_k, qT[:, h, 0:512],
                             start=True, stop=True)
            nc.tensor.matmul(sc[:, 512:1024], lhs_k, qT[:, h, 512:1024],
                             start=True, stop=True)
            return sc

        # two heads interleaved: the scores->exp->sc-reuse dependency cycle
        # then spans two iterations of independent work, halving its
        # per-iteration latency cost (PSUM: 2 sc bufs + 2 o_ps bufs = 8 banks)
        for hp in range(0, HG, 2):
            hs = (hp, hp + 1)
            o_ps = {h: pso.tile([128, S], F32, tag="o_ps", name=f"o_ps{h}")
                    for h in hs}
            accs = {h: rs_p.tile([128, S], BF16, tag="acc", name=f"acc{h}")
                    for h in hs}
            at_prev = {}
            sc_cur = {hs[0]: emit_scores(hs[0], 0), hs[1]: emit_scores(hs[1], 0)}
            for m in range(NT):
                for h in hs:
                    sc_next = emit_scores(h, m + 1) if m + 1 < NT else None
                    at = at_p.tile([128, S], BF16, tag="at", name="at")
                    nc.scalar.activation(out=at, in_=sc_cur[h], func=AF.Exp,
                                         scale=rrk_all[:, m, h:h + 1])
                    sc_cur[h] = sc_next
                    first, last = (m == 0), (m == NT - 1)
                    v_mh = vnat[:, m, h * 128:(h + 1) * 128]
                    nc.tensor.matmul(o_ps[h][:, 0:512], v_mh, at[:, 0:512],
                                     start=first, stop=last)
                    nc.tensor.matmul(o_ps[h][:, 512:1024], v_mh, at[:, 512:1024],
                                     start=first, stop=last)
                    # denominator accumulation: bf16 adds on DVE (2x mode)
                    with nc.allow_low_precision(reason="bf16 softmax denom"):
                        if m == 1:
                            nc.vector.tensor_add(out=accs[h], in0=at_prev[h],
                                                 in1=at)
                        elif m > 1:
                            nc.vector.tensor_add(out=accs[h], in0=accs[h],
                                                 in1=at)
                    at_prev[h] = at
                    if m == NT - 1:
                        # normalize this head NOW: the first head's oT-mul
                        # frees its o_ps buffer for the next pair one
                        # iteration earlier than the second head's
                        sums_b = rs_p.tile([128, S], F32, tag="sums_b",
                                           name=f"sums{h}")
                        nc.gpsimd.partition_all_reduce(sums_b, accs[h], 128,
                                                       bass_isa.ReduceOp.add)
                        nc.vector.reciprocal(out=sums_b, in_=sums_b)
                        nc.vector.tensor_mul(out=oT[:, h, :], in0=o_ps[h],
                                             in1=sums_b)

        # ---- phase F: out projection, emitted inside E's pools (po reuses the
        # sc psum tag; ot_t lives in rs_p) so no pool barrier splits E and F
        def emit_F_mm(po, m, kb):
            first, last = (kb == 0), (kb == HG - 1)
            nc.tensor.matmul(po[:, 0:512], wo_sb[:, m, kb, :], oT[:, kb, 0:512],
                             start=first, stop=last)
            nc.tensor.matmul(po[:, 512:1024], wo_sb[:, m, kb, :],
                             oT[:, kb, 512:1024], start=first, stop=last)

        def emit_F_evac(po, m):
            ot_t = rs_p.tile([128, S], F32, tag="ot_t", name=f"ot_t{m}")
            if m == KT - 1:
                # the kernel's tail: halve the evac+store chain so the first
                # half's DMA overlaps the second half's evacuation
                for g in (0, 1):
                    sl = slice(g * 512, (g + 1) * 512)
                    nc.vector.tensor_scalar(out=ot_t[:, sl], in0=po[:, sl],
                                            scalar1=gate_sb[:, m:m + 1],
                                            scalar2=vb_sb[:, m:m + 1],
                                            op0=ALU.mult, op1=ALU.add)
                    nc.sync.dma_start(out=out_t[m * 128:(m + 1) * 128, sl],
                                      in_=ot_t[:, sl])
                return
            if m % 2 == 0:
                nc.scalar.activation(out=ot_t, in_=po, func=AF.Identity,
                                     bias=vb_sb[:, m:m + 1],
                                     scale=gate_sb[:, m:m + 1])
            else:
                nc.vector.tensor_scalar(out=ot_t, in0=po,
                                        scalar1=gate_sb[:, m:m + 1],
                                        scalar2=vb_sb[:, m:m + 1],
                                        op0=ALU.mult, op1=ALU.add)
            nc.sync.dma_start(out=out_t[m * 128:(m + 1) * 128, :], in_=ot_t)

        def alloc_po(m):
            # alternate between the two dead attention psum pools for slack
            return (pssc.tile([128, S], F32, tag="sc", name=f"po{m}")
                    if m % 2 == 0
                    else pso.tile([128, S], F32, tag="o_ps", name=f"po{m}"))

        # first two output tiles interleave their head loops so F has 12
        # dense matmuls queued before it needs the last pair's normalization
        po01 = {m: alloc_po(m) for m in (0, 1)}
        for m in (0, 1):
            for kb in range(HG - 2):
                emit_F_mm(po01[m], m, kb)
        for m in (0, 1):
            for kb in (HG - 2, HG - 1):
                emit_F_mm(po01[m], m, kb)
            emit_F_evac(po01[m], m)
        for m in range(2, KT):
            po = alloc_po(m)
            for kb in range(HG):
                emit_F_mm(po, m, kb)
            emit_F_evac(po, m)

        pso_cm.__exit__(None, None, None)
        pssc_cm.__exit__(None, None, None)
        rs_cm.__exit__(None, None, None)
        at_cm.__exit__(None, None, None)
        qkT_cm.__exit__(None, None, None)
        natqk_cm.__exit__(None, None, None)
        v_cm.__exit__(None, None, None)
        wo_cm.__exit__(None, None, None)
        oT_cm.__exit__(None, None, None)
        misc_cm.__exit__(None, None, None)

    nc.compile()
    return nc


_NC_CACHE = {}


def _get_nc(has_qkv_bias, has_norm_w):
    key = (has_qkv_bias, has_norm_w)
    if key not in _NC_CACHE:
        _NC_CACHE[key] = build_nc(*key)
    return _NC_CACHE[key]


# per-head column permutation de-interleaving rope pairs:
# [0,2,..,62, 1,3,..,63, 64..127] within each head's 128 columns
def _qk_perm():
    base = np.concatenate([np.arange(0, RD, 2), np.arange(1, RD, 2),
                           np.arange(RD, HD)])
    return np.concatenate([h * HD + base for h in range(HG)])


_PERM = _qk_perm()


def prep_in_maps(x, mod, cos, sin, qkv_w, qkv_b, mod_w, mod_b, out_w, out_b,
                 norm_q_w, norm_k_w):
    """Host-side sharding. Returns (in_maps, flags, x_np)."""
    x = np.asarray(x, dtype=np.float32)
    m3 = np.asarray(mod, np.float32) @ np.asarray(mod_w, np.float32) \
        + np.asarray(mod_b, np.float32)
    bias, scale, gatef = np.split(m3, 3, axis=-1)          # [B, D] each
    scale1p = (1.0 + scale).astype(np.float32)
    vbf = (np.asarray(out_b, np.float32)[None, :] * gatef).astype(np.float32)

    qkv_b = np.asarray(qkv_b, np.float32)
    has_qkv_bias = bool(np.any(qkv_b != 0.0))
    has_norm_w = not (np.allclose(norm_q_w, 1.0) and np.allclose(norm_k_w, 1.0))

    cosb = np.ascontiguousarray(np.asarray(cos, np.float32).astype(ml_dtypes.bfloat16))
    sinb = np.ascontiguousarray(np.asarray(sin, np.float32).astype(ml_dtypes.bfloat16))
    qkv_w = np.asarray(qkv_w, np.float32)
    out_w = np.asarray(out_w, np.float32)

    # per-head rope-pair de-interleave permutation (shared by q and k; inner
    # products and rms are invariant; cos/sin indices line up with x0/x1 halves)
    perm = _PERM
    hd_perm = np.concatenate([np.arange(0, RD, 2), np.arange(1, RD, 2),
                              np.arange(RD, HD)])

    in_maps = []
    for c in range(N_CORES):
        b, g = divmod(c, 2)
        lo = g * GCOLS
        wq_c = qkv_w[:, lo:lo + GCOLS][:, perm]
        wk_c = qkv_w[:, 2048 + lo:2048 + lo + GCOLS][:, perm]
        wv_c = qkv_w[:, 4096 + lo:4096 + lo + GCOLS]
        wo_c = out_w[lo:lo + GCOLS, :]                      # [GCOLS, D]
        # wo_l[p, m, kb, c] = wo_c[kb*128+p, m*128+c]
        wo_l = np.transpose(
            wo_c.reshape(HG, 128, KT, 128), (1, 2, 0, 3))
        im = {
            "x": np.ascontiguousarray(x[b].astype(ml_dtypes.bfloat16)),
            "cos": cosb, "sin": sinb,
            "wq": np.ascontiguousarray(wq_c.astype(ml_dtypes.bfloat16)),
            "wk": np.ascontiguousarray(wk_c.astype(ml_dtypes.bfloat16)),
            "wv": np.ascontiguousarray(wv_c.astype(ml_dtypes.bfloat16)),
            "wo": np.ascontiguousarray(wo_l.astype(ml_dtypes.bfloat16)),
            "scale1p": np.ascontiguousarray(scale1p[b].reshape(KT, 128).T),
            "biasm": np.ascontiguousarray(bias[b].reshape(KT, 128).T),
            "gate": np.ascontiguousarray(gatef[b].reshape(KT, 128).T),
            "vb": np.ascontiguousarray(
                (vbf[b] if g == 0 else np.zeros_like(vbf[b])).reshape(KT, 128).T),
        }
        if has_qkv_bias:
            im["bq"] = np.ascontiguousarray(qkv_b[lo:lo + GCOLS][perm])
            im["bk"] = np.ascontiguousarray(qkv_b[2048 + lo:2048 + lo + GCOLS][perm])
            im["bv"] = np.ascontiguousarray(qkv_b[4096 + lo:4096 + lo + GCOLS])
        if has_norm_w:
            im["wqn"] = np.ascontiguousarray(
                np.asarray(norm_q_w, np.float32)[hd_perm].astype(ml_dtypes.bfloat16))
            im["wkn"] = np.ascontiguousarray(
                np.asarray(norm_k_w, np.float32)[hd_perm].astype(ml_dtypes.bfloat16))
        in_maps.append(im)
    return in_maps, (has_qkv_bias, has_norm_w), x


def gather(results, x):
    B = x.shape[0]
    outs = []
    for b in range(B):
        p = results[2 * b]["out_t"] + results[2 * b + 1]["out_t"]   # [D, S]
        outs.append(p.T + x[b])
    return np.stack(outs).astype(np.float32)


def kernel(**inputs) -> np.ndarray:
    in_maps, flags, x = prep_in_maps(**inputs)
    nc = _get_nc(*flags)
    res = run_bass_kernel_spmd(nc, in_maps, core_ids=list(range(N_CORES)))
    return gather(res.results, x)


if __name__ == "__main__":
    import time
    t0 = time.time()
    nc = build_nc(False, False)
    print("build+compile ok in", time.time() - t0, "s")
